# revision 1
# baseline (speedup 1.0000x reference)
"""Causal multi-head attention (B=2, T=2048, C=1024, H=16, D=64) on 8 trn2 cores.

Sharding: core c -> (batch c//4, head-group c%4 of 4 heads / 256 channels).
Each core computes q/k/v for its head group, causal attention, and a partial
output projection y_part[2048,1024] = attnout_g @ wo_g.T. The host sums the 4
per-group partials of each batch (the "all-reduce after wo" done host-side).

Device kernel (per core, SPMD identical program), bf16 datapath (PE rate is
the same as f32r; bf16 halves SBUF/DMA and enables FWL weight loads):
  phase B: xT,[wq|wk|wv]T loaded as bf16; q^T,k^T (head-channel major, bf16)
           and v (natural, with a ones column -> vaug) via PE matmuls.
  phase C: per head, per tq-tile(512): ST[tk128,tq512] = k^T.T @ q^T; causal
           staircase mask applied INSIDE the score psum group as an extra
           matmul (triA.T @ bsh adds -98304 to invalid slots); P = exp(ST/8)
           on ScalarE (PSUM->SBUF bf16); PV accumulated over tk into
           PSUM[65,512] where row 64 (ones column of vaug) is the softmax
           denominator; normalize with approx-reciprocal + K=1 bcast matmul.
  phase D: y[t,1024] = attnoutT.T @ woT, DMA'd out (bf16) per 256-row tile.

All tile pools live OUTSIDE the timing loop so iteration n+1's input DMAs
overlap iteration n's attention (no SBUF-region reuse between phases).
"""
import time
import hashlib
import numpy as np
import ml_dtypes

import jax
import jax.numpy as jnp
from jax.sharding import Mesh, PartitionSpec
from jax.experimental.shard_map import shard_map

import concourse.bass as bass
import concourse.tile as tile
from concourse import bacc, mybir
from concourse import bass2jax
from concourse.bass2jax import _bass_exec_p, install_neuronx_cc_hook, partition_id_tensor

B, T, C = 2, 2048, 1024
H = 16
D = C // H            # 64
SCALE = D ** -0.5     # 0.125
N_CORES = 8
HG = H // (N_CORES // B)   # heads per core = 4
HC = HG * D                # channels per core = 256
KT = C // 128              # 8 contraction tiles
NT = T // 128              # 16 row tiles
NJ = T // 512              # 4 tq tiles
F32 = mybir.dt.float32
F32R = mybir.dt.float32r
BF16 = mybir.dt.bfloat16


# ---------------------------------------------------------------- device code

def _build_nc(block_info, n_uniq, loop_n=None, phases="BCD", cast_dma=False):
    """block_info[j][i] = None (skip) | -1 (full) | ("st", slot) (causal
    staircase applied as a PE matmul accumulation) | ("mk", idx>=0) (general
    mask tile multiplied on DVE)."""
    n_shift = 1 + max([e[1] for row in block_info for e in row
                       if isinstance(e, tuple) and e[0] == "st"], default=-1)
    nc = bacc.Bacc("TRN2", target_bir_lowering=False, debug=False,
                   num_devices=N_CORES)
    xT_ap = nc.dram_tensor("xT", [C, T], BF16, kind="ExternalInput").ap()
    # weights host-packed so each SBUF partition's bytes are contiguous in
    # DRAM (large descriptors; scattered 1KB descriptors measured ~4x slower)
    wqT_ap = nc.dram_tensor("wqT", [128, KT * HC], BF16, kind="ExternalInput").ap()
    wkT_ap = nc.dram_tensor("wkT", [128, KT * HC], BF16, kind="ExternalInput").ap()
    wvT_ap = nc.dram_tensor("wvT", [128, KT * HC], BF16, kind="ExternalInput").ap()
    woT_ap = nc.dram_tensor("woT", [128, 2 * C], BF16, kind="ExternalInput").ap()
    if n_uniq > 0:
        mk_ap = nc.dram_tensor("mk", [128, n_uniq * 512], F32,
                               kind="ExternalInput").ap()
    if n_shift > 0:
        triA_ap = nc.dram_tensor("triA", [128, 128], BF16,
                                 kind="ExternalInput").ap()
        bsh_ap = nc.dram_tensor("bsh", [128, n_shift * 512], BF16,
                                kind="ExternalInput").ap()
    y_ap = nc.dram_tensor("y", [T, C], BF16, kind="ExternalOutput").ap()

    with tile.TileContext(nc) as tc:
        with (
            nc.allow_low_precision(reason="bf16 matmul pipeline"),
            tc.tile_pool(name="glob", bufs=1) as pg,
            tc.tile_pool(name="warm", bufs=1) as pwarm,
            tc.tile_pool(name="ab", bufs=1) as pab,
            tc.tile_pool(name="wp", bufs=3) as pwp,
            tc.tile_pool(name="cd", bufs=1) as pcd,
            tc.tile_pool(name="pt", bufs=4) as ppt,
            tc.tile_pool(name="small", bufs=4) as psm,
            tc.tile_pool(name="ys", bufs=3) as pys,
            # PSUM: 2x [128,2,512] (4 banks) + 4x [128,512] (4 banks)
            tc.tile_pool(name="pst", bufs=2, space="PSUM") as pst,
            tc.tile_pool(name="ppv", bufs=4, space="PSUM") as ppv,
        ):
            # persistent across phases
            qT = pg.tile([128, 2, T], BF16)        # [o-part, o-tile, t]
            # k^T zero-padded per head: head h lives in partition rows
            # 64*(h%2)..+64 of kTz[:, h, :], other rows stay 0 so the score
            # matmul contracts K=128 against the 2-head-packed qT.
            kTz = pg.tile([128, HG, T], BF16)
            # v natural per tk-tile/head, padded to 128 cols: [v | 1 | zeros]
            # (128 cols keeps FWL weight loads; ones col gives softmax denom)
            vaug = pg.tile([128, NT, HG, 128], BF16)
            ones128 = pg.tile([128, 128], BF16)    # all-ones lhsT for bcast
            recipz = pg.tile([128, 512], BF16)     # row0=recip, rows1-127 zero
            ident = pg.tile([128, 128], F32)       # PE-transpose identity
            if n_shift > 0:
                triA = pg.tile([128, 128], BF16)
                bsh = pg.tile([128, n_shift, 512], BF16)
                nc.sync.dma_start(triA[:], triA_ap)
                nc.sync.dma_start(
                    bsh[:], bsh_ap.rearrange("p (s f) -> p s f", f=512))

            # warm the Exp table while DMAs run; zeros/ones are produced via
            # casting copies from f32 tiles (works for every dtype).
            wtile = pwarm.tile([1, 16], F32)
            nc.vector.memset(wtile[:], 0.0)
            nc.scalar.activation(wtile[:], wtile[:],
                                 mybir.ActivationFunctionType.Exp)
            zt = pwarm.tile([128, 512], F32)
            nc.vector.memset(zt[:], 0.0)
            nc.vector.tensor_copy(kTz[:].rearrange("p h (j f) -> p h j f", f=512), zt[:, None, None, :].broadcast_to([128, HG, NJ, 512]))
            nc.vector.tensor_copy(vaug[:], zt[:, None, None, 0:128].broadcast_to([128, NT, HG, 128]))
            nc.vector.tensor_copy(recipz[:], zt[:])
            import concourse.masks as _masks
            _masks.make_identity(nc, ident[:])
            ot = pwarm.tile([128, 16], F32)
            nc.vector.memset(ot[:], 1.0)
            nc.vector.tensor_copy(vaug[:, :, :, D], ot[:, None, 0:HG].broadcast_to([128, NT, HG]))
            nc.vector.tensor_copy(ones128[:], ot[:, 0:1].broadcast_to([128, 128]))

            # persistent phase tiles (hoisted so the loop can pipeline)
            xT = pab.tile([128, KT, T], BF16)
            vT = pab.tile([128, 2, T], F32R)
            woT = pcd.tile([128, 2, C], BF16)
            attnoutT = pcd.tile([128, 2, T], BF16)
            if n_uniq > 0:
                mks = pcd.tile([128, n_uniq, 512], F32)

            def body(it):
                # ---------------- phase A/B: loads + projections ------------
                # DMA issue order puts wq-half0 + x-chunk0 first so the first
                # projection matmul starts ~3us after the previous body ends
                xr = xT_ap.rearrange("(k p) t -> k p t", p=128)
                w_ts = [pwp.tile([128, KT, HC], BF16, tag="w",
                                 name=f"w{qk}_{it}") for qk in range(3)]
                wqr = wqT_ap.rearrange("p (k m) -> p k m", k=KT)
                nc.sync.dma_start(w_ts[0][:, 0:KT // 2, :], wqr[:, 0:KT // 2, :])
                nc.sync.dma_start(xT[:, 0, :], xr[0])
                nc.sync.dma_start(w_ts[0][:, KT // 2:, :], wqr[:, KT // 2:, :])
                nc.sync.dma_start(xT[:, 1, :], xr[1])
                nc.sync.dma_start(w_ts[1][:], wkT_ap.rearrange("p (k m) -> p k m", k=KT))
                for kc in range(2, KT):
                    nc.sync.dma_start(xT[:, kc, :], xr[kc])
                nc.sync.dma_start(w_ts[2][:], wvT_ap.rearrange("p (k m) -> p k m", k=KT))
                nc.sync.dma_start(woT[:], woT_ap.rearrange("p (k m) -> p k m", k=2))
                if n_uniq > 0:
                    nc.sync.dma_start(mks[:], mk_ap.rearrange("p (u f) -> p u f", f=512))
                if "B" not in phases:
                    return

                # q^T, k^T, v^T: [o, t] = w_g @ x^T; kc-inner per j-pair so
                # each psum pair drains while the next pair accumulates
                for qk in range(3):
                    w_t = w_ts[qk]
                    for m in range(2):
                        for jh in range(2):
                            pss = pst.tile([128, 2, 512], F32, tag="st",
                                           name=f"qkps{qk}_{m}_{jh}_{it}")
                            for kc in range(KT):
                                for j2 in range(2):
                                    j = 2 * jh + j2
                                    nc.tensor.matmul(
                                        pss[:, j2, :],
                                        w_t[:, kc, 128 * m:128 * (m + 1)],
                                        xT[:, kc, 512 * j:512 * (j + 1)],
                                        start=(kc == 0), stop=(kc == KT - 1))
                            for j2 in range(2):
                                j = 2 * jh + j2
                                sl = slice(512 * j, 512 * (j + 1))
                                if qk == 0:
                                    nc.scalar.copy(qT[:, m, sl], pss[:, j2, :])
                                elif qk == 2:
                                    nc.scalar.copy(vT[:, m, sl], pss[:, j2, :])
                                else:
                                    # scatter psum head-halves into kTz rows
                                    nc.scalar.copy(kTz[0:64, 2 * m, sl],
                                                   pss[0:64, j2, :])
                                    nc.scalar.copy(kTz[64:128, 2 * m + 1, sl],
                                                   pss[64:128, j2, :])
                # v natural via PE transpose of vT 128x128 blocks
                for m in range(2):
                    for i in range(NT):
                        ps = ppv.tile([128, 512], F32, tag="pv", name=f"vtp{m}_{i}_{it}")
                        nc.tensor.transpose(
                            ps[:, 0:128], vT[:, m, 128 * i:128 * (i + 1)].bitcast(F32),
                            ident[:])
                        nc.vector.tensor_copy(
                            vaug[:, i, 2 * m:2 * m + 2, 0:D],
                            ps[:, 0:128].rearrange("p (h d) -> p h d", h=2))

                # -------- phase C+D: attention, interleaved with out-proj ----
                if "C" not in phases and "D" not in phases:
                    return
                for j in range(NJ):
                    blocks = [(i, bi) for i, bi in enumerate(block_info[j])
                              if bi is not None]
                    chunks = [blocks[c:c + 2] for c in range(0, len(blocks), 2)]
                    for h in range(HG if "C" in phases else 0):
                        m = h // 2
                        jsl = slice(512 * j, 512 * (j + 1))
                        pv = ppv.tile([128, 512], F32, tag="pv", name=f"pv{h}_{j}_{it}")
                        n_acc = len(blocks)
                        acc = 0
                        prev_chunk = None  # (pt, idxs)

                        def emit_pv(pt, idxs):
                            nonlocal acc
                            for c, i in enumerate(idxs):
                                nc.tensor.matmul(
                                    pv[:], vaug[:, i, h, :], pt[:, c, :],
                                    start=(acc == 0), stop=(acc == n_acc - 1))
                                acc += 1

                        for ch in chunks:
                            nsub = len(ch)
                            st = pst.tile([128, 2, 512], F32, tag="st", name=f"st{h}_{j}_{it}")
                            for c, (i, bi) in enumerate(ch):
                                stair = (isinstance(bi, tuple)
                                         and bi[0] == "st")
                                nc.tensor.matmul(
                                    st[:, c, :],
                                    kTz[:, h, 128 * i:128 * (i + 1)],
                                    qT[:, m, jsl],
                                    start=True, stop=not stair)
                                if stair:
                                    # add -98304 to causally-invalid slots
                                    nc.tensor.matmul(
                                        st[:, c, :], triA[:],
                                        bsh[:, bi[1], :],
                                        start=False, stop=True)
                            pt = ppt.tile([128, 2, 512], BF16, tag="pt")
                            # one exp per chunk: the fixed per-op ACT cost
                            # (352 cyc) amortizes over 1024 elements
                            nc.scalar.activation(
                                pt[:, 0:nsub, :], st[:, 0:nsub, :],
                                mybir.ActivationFunctionType.Exp, scale=SCALE)
                            for c, (_, b) in enumerate(ch):
                                if isinstance(b, tuple) and b[0] == "mk":
                                    nc.vector.tensor_mul(
                                        pt[:, c, :], pt[:, c, :],
                                        mks[:, b[1], :])
                            if prev_chunk is not None:
                                emit_pv(*prev_chunk)
                            prev_chunk = (pt, [i for i, _ in ch])
                        emit_pv(*prev_chunk)
                        # normalization: 1/denom (row 64) broadcast down 128
                        # partitions via ones-column matmul; fast approx
                        # recip (18 bits) wants an SBUF f32 operand
                        dnm = psm.tile([1, 512], F32, tag="dnm")
                        nc.vector.tensor_copy(dnm[:], pv[64:65, :])
                        recip = psm.tile([1, 512], F32, tag="recip")
                        nc.vector.reciprocal_approx_fast(recip[:], dnm[:])
                        nc.vector.tensor_copy(recipz[0:1, :], recip[:])
                        bc = ppv.tile([128, 512], F32, tag="pv", name=f"bc{h}_{j}_{it}")
                        nc.tensor.matmul(bc[:], ones128[:], recipz[:],
                                         start=True, stop=True)
                        avu = psm.tile([64, 512], F32, tag="avu")
                        nc.vector.tensor_copy(avu[:], pv[0:64, :])
                        row = 64 * (h % 2)
                        nc.vector.tensor_mul(
                            attnoutT[row:row + 64, m, jsl],
                            avu[:], bc[0:64, :])

                    # ---- phase D for this j: y rows [512j, 512j+512) ----
                    if "D" not in phases:
                        continue
                    for tp in range(2):     # pairs of row tiles
                        ys = pys.tile([128, 2, C], BF16, tag="ys")
                        for tsub in range(2):
                            t = 4 * j + 2 * tp + tsub
                            for o2 in range(2):
                                ps = ppv.tile([128, 512], F32, tag="pv", name=f"yps{t}_{o2}_{it}")
                                for kc in range(2):
                                    nc.tensor.matmul(
                                        ps[:],
                                        attnoutT[:, kc, 128 * t:128 * (t + 1)],
                                        woT[:, kc, 512 * o2:512 * (o2 + 1)],
                                        start=(kc == 0), stop=(kc == 1))
                                nc.vector.tensor_copy(
                                    ys[:, tsub, 512 * o2:512 * (o2 + 1)], ps[:])
                        r0 = 512 * j + 256 * tp
                        nc.scalar.dma_start(
                            y_ap[r0:r0 + 256, :].rearrange("(tt p) o -> p tt o", p=128),
                            ys[:])

            if loop_n is None:
                body(0)
            else:
                # unrolled loop body: no all-engine reset barrier between the
                # unrolled copies, so copy n+1's input DMAs overlap copy n
                unroll = 4 if loop_n % 4 == 0 else (2 if loop_n % 2 == 0 else 1)
                with tc.For_i(0, loop_n // unroll, 1, staggered_reset=True):
                    for it in range(unroll):
                        body(it)

    nc.compile()
    return nc


# ---------------------------------------------------------------- run harness

def _install_verbose_hook():
    install_neuronx_cc_hook()
    try:
        import libneuronxla
    except ImportError:
        return
    import traceback
    inner = bass2jax.neuronx_cc_hook

    def wrapped(*a, **kw):
        try:
            return inner(*a, **kw)
        except BaseException:
            traceback.print_exc()
            raise
    libneuronxla.neuronx_cc = wrapped


class _SpmdRunner:
    def __init__(self, nc, n_cores):
        _install_verbose_hook()
        self.nc, self.n_cores = nc, n_cores
        pname = nc.partition_id_tensor.name if nc.partition_id_tensor else None
        in_names, out_names, out_avals = [], [], []
        for alloc in nc.m.functions[0].allocations:
            if not isinstance(alloc, mybir.MemoryLocationSet):
                continue
            name = alloc.memorylocations[0].name
            if alloc.kind == "ExternalInput":
                if name != pname:
                    in_names.append(name)
            elif alloc.kind == "ExternalOutput":
                out_names.append(name)
                out_avals.append(jax.core.ShapedArray(
                    tuple(alloc.tensor_shape), mybir.dt.np(alloc.dtype)))
        self.in_names, self.out_names, self.out_avals = in_names, out_names, out_avals
        n_params = len(in_names)
        all_in = list(in_names) + list(out_names)
        if pname is not None:
            all_in.append(pname)

        def _body(*args):
            operands = list(args)
            if pname is not None:
                operands.append(partition_id_tensor())
            return tuple(_bass_exec_p.bind(
                *operands,
                out_avals=tuple(out_avals), in_names=tuple(all_in),
                out_names=tuple(out_names), lowering_input_output_aliases=(),
                sim_require_finite=True, sim_require_nnan=True, nc=nc))

        devices = jax.devices()[:n_cores]
        self.mesh = Mesh(np.asarray(devices), ("core",))
        in_specs = (PartitionSpec("core"),) * (n_params + len(out_names))
        out_specs = (PartitionSpec("core"),) * len(out_names)
        self.fn = jax.jit(shard_map(_body, mesh=self.mesh, in_specs=in_specs,
                                    out_specs=out_specs, check_rep=False),
                          keep_unused=True)
        self._shard = jax.sharding.NamedSharding(self.mesh, PartitionSpec("core"))

    def put_inputs(self, in_maps):
        arrs = []
        for name in self.in_names:
            cat = np.concatenate([np.asarray(m[name]) for m in in_maps], axis=0)
            arrs.append(jax.device_put(cat, self._shard))
        for av in self.out_avals:
            z = np.zeros((self.n_cores * av.shape[0], *av.shape[1:]), av.dtype)
            arrs.append(jax.device_put(z, self._shard))
        return arrs

    def run(self, dev_args):
        outs = self.fn(*dev_args)
        jax.block_until_ready(outs)
        return outs

    def results(self, outs):
        per_core = []
        for c in range(self.n_cores):
            per_core.append({
                name: np.asarray(outs[i]).reshape(
                    self.n_cores, *self.out_avals[i].shape)[c]
                for i, name in enumerate(self.out_names)})
        return per_core


# ---------------------------------------------------------------- host side

def _mask_blocks(mask):
    """Classify transposed 128x512 blocks of the [T,T] mask.

    Returns (block_info, uniq, shifts) where block_info[j][i] is None (all
    masked), -1 (all valid), ("st", slot) (causal staircase valid = p <=
    f - shifts[slot], applied on-device as a matmul accumulation), or
    ("mk", idx) (arbitrary mixed pattern, multiplied from uniq[idx])."""
    m2 = np.asarray(mask).reshape(T, T)
    valid = (m2 != -np.inf)          # [tq, tk]
    validT = valid.T                 # [tk, tq]
    uniq, keys = [], {}
    shifts, shift_keys = [], {}
    p_idx = np.arange(128)[:, None]
    f_idx = np.arange(512)[None, :]
    block_info = []
    for j in range(NJ):
        row = []
        for i in range(NT):
            blk = validT[128 * i:128 * (i + 1), 512 * j:512 * (j + 1)]
            if not blk.any():
                row.append(None)
                continue
            if blk.all():
                row.append(-1)
                continue
            s = 128 * i - 512 * j
            if -512 < s < 512 and np.array_equal(blk, p_idx <= f_idx - s):
                if s not in shift_keys:
                    shift_keys[s] = len(shifts)
                    shifts.append(s)
                row.append(("st", shift_keys[s]))
                continue
            k = hashlib.sha1(np.ascontiguousarray(blk)).hexdigest()
            if k not in keys:
                keys[k] = len(uniq)
                uniq.append(blk.astype(np.float32))
            row.append(("mk", keys[k]))
        block_info.append(row)
    return block_info, uniq, shifts


_MASK_BIG = -98304.0    # -1.5 * 2**16, exact in bf16


def _stair_operands(shifts):
    """triA [128,128] and bsh [128, n_shift, 512] with
    (triA.T @ bsh[:, slot])[p, f] = _MASK_BIG where p > f - shifts[slot]."""
    k_i = np.arange(128)
    p_i = np.arange(128)
    triA = ((p_i[None, :] > k_i[:, None]) | (k_i[:, None] == 127)
            ).astype(np.float32)
    bsh = np.zeros((128, len(shifts), 512), np.float32)
    for slot, s in enumerate(shifts):
        for f in range(512):
            g = f - s
            if g < 0:
                bsh[127, slot, f] = _MASK_BIG
            elif g <= 126:
                bsh[g, slot, f] = _MASK_BIG
    return triA, bsh


_CACHE = {}


def _get_runner(block_info, n_uniq, loop_n=None, phases="BCD", cast_dma=True):
    key = (str(block_info), n_uniq, loop_n, phases, cast_dma)
    if key not in _CACHE:
        nc = _build_nc(block_info, n_uniq, loop_n=loop_n, phases=phases, cast_dma=cast_dma)
        _CACHE[key] = _SpmdRunner(nc, N_CORES)
    return _CACHE[key]


def _bf16(a):
    return np.ascontiguousarray(np.asarray(a, np.float32)).astype(
        ml_dtypes.bfloat16)


def _pack_rows(a):
    """[R*128, F] -> [128, R*F]: partition-contiguous packing for fast DMA."""
    r = a.shape[0] // 128
    return np.ascontiguousarray(
        a.reshape(r, 128, a.shape[1]).transpose(1, 0, 2).reshape(128, -1))


def _make_in_maps(x, mask, wq, wk, wv, wo):
    block_info, uniq, shifts = _mask_blocks(mask)
    x = np.asarray(x, np.float32)
    extra = {}
    if uniq:
        mk = np.stack(uniq)    # [u,128,512] -> [128, u*512]
        extra["mk"] = np.ascontiguousarray(
            mk.transpose(1, 0, 2).reshape(128, -1))
    if shifts:
        triA, bsh = _stair_operands(shifts)
        extra["triA"] = _bf16(triA)
        extra["bsh"] = _bf16(np.ascontiguousarray(bsh.reshape(128, -1)))
    in_maps = []
    for c in range(N_CORES):
        b, g = c // 4, c % 4
        sl = slice(HC * g, HC * (g + 1))
        in_maps.append({
            "xT": _bf16(x[b].T),
            "wqT": _pack_rows(_bf16(np.asarray(wq)[sl, :].T)),
            "wkT": _pack_rows(_bf16(np.asarray(wk)[sl, :].T)),
            "wvT": _pack_rows(_bf16(np.asarray(wv)[sl, :].T)),
            "woT": _pack_rows(_bf16(np.asarray(wo)[:, sl].T)),
            **extra,
        })
    return in_maps, block_info, len(uniq)


def kernel(x, mask, wq, wk, wv, wo):
    in_maps, block_info, n_uniq = _make_in_maps(x, mask, wq, wk, wv, wo)
    runner = _get_runner(block_info, n_uniq)
    dev = runner.put_inputs(in_maps)
    res = runner.results(runner.run(dev))
    out = np.zeros((B, T, C), np.float32)
    for c in range(N_CORES):
        out[c // 4] += res[c]["y"].astype(np.float32)
    return out



# revision 20
# speedup vs baseline: 1.0155x; 1.0155x over previous
"""Causal multi-head attention (B=2, T=2048, C=1024, H=16, D=64) on 8 trn2 cores.

Sharding: core c -> (batch c//4, head-group c%4 of 4 heads / 256 channels).
Each core computes q/k/v for its head group, causal attention, and a partial
output projection y_part[2048,1024] = attnout_g @ wo_g.T. The host sums the 4
per-group partials of each batch (the "all-reduce after wo" done host-side).

Device kernel (per core, SPMD identical program), bf16 datapath:
  phase B: xT,[wq|wk]T loaded bf16; q^T,k^T packed 2-heads-per-128-partitions
           (head h in rows 64*(h%2)..+64 of slot h//2); scores contract K=64
           directly on those slices, so k needs no zero-padding. v computed
           NATURALLY (v[t,o] = xT-slice.T @ wvT) into vaug[tk, i, h, 0:64]
           with a ones column at 64 (softmax denominator via PV matmul).
  phase C: per head/tq-chunk(512): ST[tk128,tq512] = kT.T @ qT (K=64); P =
           exp(ST/8) on ScalarE -> bf16; causal staircase applied as a bf16
           mask MULTIPLY on DVE (only cols >= 128r are ever read); PV is
           FLIPPED: out[tq128, 65] = P-slice.T @ vaug (F=65, 4 tq-slices
           per chunk accumulated in one [128,4,66] psum bank); col 64 is the
           denominator, per-PARTITION, so normalization is one approx-recip
           + tensor_scalar_mul per tq-slice. attnout natural [tq, ch] is
           PE-transposed back to attnoutT[ch, t] for the out-projection.
  phase D: y[t,1024] = attnoutT.T @ woT per 128-row tile; psum->sbuf copies
           split over DVE/Pool; DMA out on the Pool queue. D(j) work is
           queued as PE gap-fillers into phase C of the next tq-chunk.

All per-iteration tiles come from bufs>=2 pools so iteration n+1's DMAs and
projections overlap iteration n's attention tail.
"""
import hashlib
import numpy as np
import ml_dtypes
from collections import deque

import jax
import jax.numpy as jnp
from jax.sharding import Mesh, PartitionSpec
from jax.experimental.shard_map import shard_map

import concourse.bass as bass
import concourse.tile as tile
from concourse import bacc, mybir
from concourse import bass2jax
from concourse.bass2jax import _bass_exec_p, install_neuronx_cc_hook, partition_id_tensor

B, T, C = 2, 2048, 1024
H = 16
D = C // H            # 64
SCALE = D ** -0.5     # 0.125
N_CORES = 8
HG = H // (N_CORES // B)   # heads per core = 4
HC = HG * D                # channels per core = 256
KT = C // 128              # 8 contraction tiles
NT = T // 128              # 16 row tiles
NJ = T // 512              # 4 tq tiles
F32 = mybir.dt.float32
BF16 = mybir.dt.bfloat16
LOOKAHEAD = 2              # chunks in flight between exp and PV


# ---------------------------------------------------------------- device code

def _build_nc(block_info, n_uniq, shifts=(), loop_n=None, phases="BCD",
              cast_dma=False, dbg=False):
    """block_info[j][i] = None (skip) | -1 (full) | ("st", slot) (causal
    staircase masked by a bf16 multiply on DVE) | ("mk", idx>=0) (general
    mask tile multiplied on DVE)."""
    n_shift = len(shifts)
    # per stair slot: first tq-128-slice that reads this tile
    rmin = [max(0, -(-(s - 127) // 128)) for s in shifts]
    nc = bacc.Bacc("TRN2", target_bir_lowering=False, debug=False,
                   num_devices=N_CORES)
    xT_ap = nc.dram_tensor("xT", [C, T], BF16, kind="ExternalInput").ap()
    # weights host-packed so each SBUF partition's bytes are contiguous in
    # DRAM (large descriptors; scattered 1KB descriptors measured ~4x slower)
    wqT_ap = nc.dram_tensor("wqT", [128, KT * HC], BF16, kind="ExternalInput").ap()
    wkT_ap = nc.dram_tensor("wkT", [128, KT * HC], BF16, kind="ExternalInput").ap()
    wvT_ap = nc.dram_tensor("wvT", [128, KT * HC], BF16, kind="ExternalInput").ap()
    woT_ap = nc.dram_tensor("woT", [128, 2 * C], BF16, kind="ExternalInput").ap()
    if n_uniq > 0:
        mk_ap = nc.dram_tensor("mk", [128, n_uniq * 512], F32,
                               kind="ExternalInput").ap()
    y_ap = nc.dram_tensor("y", [T, C], BF16, kind="ExternalOutput").ap()
    if dbg:
        dqT_ap = nc.dram_tensor("dqT", [128, 2 * T], BF16, kind="ExternalOutput").ap()
        dkT_ap = nc.dram_tensor("dkT", [128, 2 * T], BF16, kind="ExternalOutput").ap()
        dva_ap = nc.dram_tensor("dva", [128, NT * HG * 66], BF16, kind="ExternalOutput").ap()
        dao_ap = nc.dram_tensor("dao", [128, 2 * T], BF16, kind="ExternalOutput").ap()

    with tile.TileContext(nc) as tc:
        with (
            nc.allow_low_precision(reason="bf16 matmul pipeline"),
            tc.tile_pool(name="glob", bufs=1) as pg,
            tc.tile_pool(name="warm", bufs=1) as pwarm,
            tc.tile_pool(name="ab", bufs=1) as pab,
            tc.tile_pool(name="wp", bufs=3) as pwp,
            tc.tile_pool(name="qk", bufs=2) as pqk,
            tc.tile_pool(name="va", bufs=2) as pva,
            tc.tile_pool(name="ao", bufs=2) as pao,
            tc.tile_pool(name="an", bufs=2) as pan,
            tc.tile_pool(name="wo", bufs=2) as pwo,
            tc.tile_pool(name="mks", bufs=2) as pmks,
            tc.tile_pool(name="pt", bufs=4) as ppt,
            tc.tile_pool(name="small", bufs=8) as psm,
            tc.tile_pool(name="ys", bufs=3) as pys,
            # PSUM: 2x [128,2,512] (4 banks) + 4x [128,512] (4 banks).
            # One matmul accumulation group per 2KB bank: a group's start
            # clobbers other partials in the same bank (measured in sim).
            tc.tile_pool(name="pst", bufs=2, space="PSUM") as pst,
            tc.tile_pool(name="ppv", bufs=4, space="PSUM") as ppv,
        ):
            ident = pg.tile([128, 128], F32)       # PE-transpose identity

            # warm the Exp table while DMAs run
            wtile = pwarm.tile([1, 16], F32)
            nc.vector.memset(wtile[:], 0.0)
            nc.scalar.activation(wtile[:], wtile[:],
                                 mybir.ActivationFunctionType.Exp)
            import concourse.masks as _masks
            _masks.make_identity(nc, ident[:])

            xT = pab.tile([128, KT, T], BF16)

            def body(it):
                # ---------------- phase A/B: loads + projections ------------
                # DMA issue order puts wq-half0 + x-chunk0 first so the first
                # projection matmul starts ~3us after the previous body's B
                xr = xT_ap.rearrange("(k p) t -> k p t", p=128)
                w_ts = [pwp.tile([128, KT, HC], BF16, tag="w",
                                 name=f"w{qk}_{it}") for qk in range(3)]
                wqr = wqT_ap.rearrange("p (k m) -> p k m", k=KT)
                nc.sync.dma_start(w_ts[0][:, 0:KT // 2, :], wqr[:, 0:KT // 2, :])
                nc.sync.dma_start(xT[:, 0, :], xr[0])
                nc.sync.dma_start(w_ts[0][:, KT // 2:, :], wqr[:, KT // 2:, :])
                nc.sync.dma_start(xT[:, 1, :], xr[1])
                nc.sync.dma_start(w_ts[1][:], wkT_ap.rearrange("p (k m) -> p k m", k=KT))
                for kc in range(2, KT):
                    nc.sync.dma_start(xT[:, kc, :], xr[kc])
                nc.sync.dma_start(w_ts[2][:], wvT_ap.rearrange("p (k m) -> p k m", k=KT))
                woT = pwo.tile([128, 2, C], BF16, tag="woT", name=f"wo_{it}")
                nc.sync.dma_start(woT[:], woT_ap.rearrange("p (k m) -> p k m", k=2))
                if n_uniq > 0:
                    mks = pmks.tile([128, n_uniq, 512], F32, tag="mks",
                                    name=f"mks_{it}")
                    nc.sync.dma_start(mks[:], mk_ap.rearrange("p (u f) -> p u f", f=512))
                if "B" not in phases:
                    return

                # q^T, k^T: [o, t] = w_g @ x^T, 2 heads per 128 partitions
                qT = pqk.tile([128, 2, T], BF16, tag="qT", name=f"qT_{it}")
                kTp = pqk.tile([128, 2, T], BF16, tag="kT", name=f"kT_{it}")
                for qk in range(2):
                    w_t = w_ts[qk]
                    dst = qT if qk == 0 else kTp
                    for m in range(2):
                        for jh in range(2):
                            pss = pst.tile([128, 2, 512], F32, tag="st",
                                           name=f"qkps{qk}_{m}_{jh}_{it}")
                            for kc in range(KT):
                                for j2 in range(2):
                                    j = 2 * jh + j2
                                    nc.tensor.matmul(
                                        pss[:, j2, :],
                                        w_t[:, kc, 128 * m:128 * (m + 1)],
                                        xT[:, kc, 512 * j:512 * (j + 1)],
                                        start=(kc == 0), stop=(kc == KT - 1))
                            for j2 in range(2):
                                j = 2 * jh + j2
                                nc.vector.tensor_copy(
                                    dst[:, m, 512 * j:512 * (j + 1)],
                                    pss[:, j2, :])
                # v natural: v[t, o] accumulated per 128-row tile
                vaug = pva.tile([128, NT, HG, 66], BF16, tag="va",
                                name=f"va_{it}")
                for i in range(NT):
                    pvn = pst.tile([128, 2, 512], F32, tag="st",
                                   name=f"vn{i}_{it}")
                    flat = pvn[:, 0, 0:HC]
                    for kc in range(KT):
                        nc.tensor.matmul(
                            flat, xT[:, kc, 128 * i:128 * (i + 1)],
                            w_ts[2][:, kc, :],
                            start=(kc == 0), stop=(kc == KT - 1))
                    nc.vector.tensor_copy(
                        vaug[:, i, :, 0:D],
                        flat.rearrange("p (h d) -> p h d", h=HG))
                nc.vector.memset(vaug[:, :, :, D], 1.0)  # denominator column

                # -------- phase C+D: attention, D(j) interleaved as fillers -
                if "C" not in phases and "D" not in phases:
                    return
                attnoutT = pao.tile([128, 2, T], BF16, tag="ao",
                                    name=f"ao_{it}")
                fillers = deque()

                def d_unit(j, tp, tsub):
                    def emit():
                        t = 4 * j + 2 * tp + tsub
                        yps = pst.tile([128, 2, 512], F32, tag="st",
                                       name=f"yps{t}_{it}")
                        for o2 in range(2):
                            for kc in range(2):
                                nc.tensor.matmul(
                                    yps[:, o2, :],
                                    attnoutT[:, kc, 128 * t:128 * (t + 1)],
                                    woT[:, kc, 512 * o2:512 * (o2 + 1)],
                                    start=(kc == 0), stop=(kc == 1))
                        ys = d_unit.ys
                        if tsub == 0:
                            ys = d_unit.ys = pys.tile(
                                [128, 2, C], BF16, tag="ys", name=f"ys{t}_{it}")
                        nc.vector.tensor_copy(
                            ys[:, tsub, :], yps[:].rearrange("p a b -> p (a b)"))
                        if tsub == 1:
                            r0 = 512 * j + 256 * tp
                            nc.gpsimd.dma_start(
                                y_ap[r0:r0 + 256, :].rearrange(
                                    "(tt p) o -> p tt o", p=128),
                                ys[:])
                    return emit
                d_unit.ys = None

                for j in range(NJ):
                    blocks = [(i, bi) for i, bi in enumerate(block_info[j])
                              if bi is not None]
                    chunks = [blocks[c:c + 2] for c in range(0, len(blocks), 2)]
                    # PV accumulation bookkeeping: contributors per tq-slice
                    contrib = [[] for _ in range(4)]
                    for i, bi in blocks:
                        lo = rmin[bi[1]] if (isinstance(bi, tuple)
                                             and bi[0] == "st") else 0
                        for rp in range(lo, 4):
                            contrib[rp].append(i)
                    anat = pan.tile([128, 2, 4, 2, D], F32, tag="an",
                                    name=f"an{j}_{it}")
                    for h in range(HG if "C" in phases else 0):
                        m, hh = h // 2, h % 2
                        r0 = 64 * hh
                        jsl = slice(512 * j, 512 * (j + 1))
                        # one [128,512] bank per tq-slice accumulation group
                        pvs = [ppv.tile([128, 512], F32, tag="pv",
                                        name=f"pv{h}_{j}_{rp}_{it}")
                               for rp in range(4)]
                        seen = [0] * 4

                        def emit_pv(pt, ch):
                            for c, (i, bi) in enumerate(ch):
                                lo = rmin[bi[1]] if (isinstance(bi, tuple)
                                                     and bi[0] == "st") else 0
                                for rp in range(lo, 4):
                                    seen[rp] += 1
                                    nc.tensor.matmul(
                                        pvs[rp][:, 0:65],
                                        pt[:, c, 128 * rp:128 * (rp + 1)],
                                        vaug[:, i, h, 0:65],
                                        start=(seen[rp] == 1),
                                        stop=(seen[rp] == len(contrib[rp])))

                        pend = deque()
                        for ch in chunks:
                            nsub = len(ch)
                            st = pst.tile([128, 2, 512], F32, tag="st",
                                          name=f"st{h}_{j}_{it}")
                            for c, (i, bi) in enumerate(ch):
                                nc.tensor.matmul(
                                    st[:, c, :],
                                    kTp[r0:r0 + 64, m, 128 * i:128 * (i + 1)],
                                    qT[r0:r0 + 64, m, jsl],
                                    start=True, stop=True)
                            pt = ppt.tile([128, 2, 512], BF16, tag="pt")
                            # one exp per chunk: fixed per-op ACT cost
                            # amortizes over 1024 elements
                            nc.scalar.activation(
                                pt[:, 0:nsub, :], st[:, 0:nsub, :],
                                mybir.ActivationFunctionType.Exp, scale=SCALE)
                            for c, (i, bi) in enumerate(ch):
                                if not isinstance(bi, tuple):
                                    continue
                                if bi[0] == "st":
                                    # causal staircase: zero exp output where
                                    # p > f - s, on the idle Pool engine
                                    s = shifts[bi[1]]
                                    c0 = 128 * rmin[bi[1]]
                                    nc.gpsimd.affine_select(
                                        out=pt[:, c, c0:],
                                        in_=pt[:, c, c0:],
                                        compare_op=mybir.AluOpType.is_ge,
                                        fill=0.0,
                                        base=c0 - s,
                                        pattern=[[1, 512 - c0]],
                                        channel_multiplier=-1)
                                else:
                                    nc.vector.tensor_mul(
                                        pt[:, c, :], pt[:, c, :],
                                        mks[:, bi[1], :])
                            pend.append((pt, ch))
                            if len(pend) > LOOKAHEAD:
                                emit_pv(*pend.popleft())
                            if fillers:
                                fillers.popleft()()
                        while pend:
                            emit_pv(*pend.popleft())

                        # normalization: per-partition recip of denominator
                        dn = psm.tile([128, 4], F32, tag="dn")
                        for rp in range(4):
                            if contrib[rp]:
                                nc.vector.tensor_copy(
                                    dn[:, rp:rp + 1], pvs[rp][:, D:D + 1])
                        rc = psm.tile([128, 4], F32, tag="rc")
                        nc.vector.reciprocal_approx_fast(rc[:], dn[:])
                        for rp in range(4):
                            if not contrib[rp]:
                                nc.vector.memset(anat[:, m, rp, hh, :], 0.0)
                                continue
                            nc.vector.tensor_scalar_mul(
                                anat[:, m, rp, hh, :], pvs[rp][:, 0:D],
                                rc[:, rp:rp + 1])

                        if hh == 1:
                            # both heads of pair m done: transpose natural
                            # attnout back to channel-major for out-proj
                            for half in range(2):
                                trp = ppv.tile([128, 512], F32, tag="pv",
                                               name=f"tr{m}_{half}_{j}_{it}")
                                for q2 in range(2):
                                    rp = 2 * half + q2
                                    nc.tensor.transpose(
                                        trp[:, 128 * q2:128 * (q2 + 1)],
                                        anat[:, m, rp, :, :].rearrange(
                                            "p a b -> p (a b)"),
                                        ident[:])
                                a0 = 512 * j + 256 * half
                                nc.vector.tensor_copy(
                                    attnoutT[:, m, a0:a0 + 256],
                                    trp[:, 0:256])

                    # ---- phase D for this j: queued as PE gap fillers ----
                    if "D" not in phases:
                        continue
                    for tp in range(2):
                        for tsub in range(2):
                            fillers.append(d_unit(j, tp, tsub))
                while fillers:
                    fillers.popleft()()
                if dbg:
                    nc.sync.dma_start(dqT_ap.rearrange("p (a t) -> p a t", a=2), qT[:])
                    nc.sync.dma_start(dkT_ap.rearrange("p (a t) -> p a t", a=2), kTp[:])
                    nc.sync.dma_start(dva_ap.rearrange("p (i h e) -> p i h e", i=NT, h=HG), vaug[:])
                    nc.sync.dma_start(dao_ap.rearrange("p (a t) -> p a t", a=2), attnoutT[:])

            if loop_n is None:
                body(0)
            else:
                # unrolled loop body: no all-engine reset barrier between the
                # unrolled copies, so copy n+1's input DMAs overlap copy n
                unroll = 4 if loop_n % 4 == 0 else (2 if loop_n % 2 == 0 else 1)
                with tc.For_i(0, loop_n // unroll, 1, staggered_reset=True):
                    for it in range(unroll):
                        body(it)

    nc.compile()
    return nc


# ---------------------------------------------------------------- run harness

def _install_verbose_hook():
    install_neuronx_cc_hook()
    try:
        import libneuronxla
    except ImportError:
        return
    import traceback
    inner = bass2jax.neuronx_cc_hook

    def wrapped(*a, **kw):
        try:
            return inner(*a, **kw)
        except BaseException:
            traceback.print_exc()
            raise
    libneuronxla.neuronx_cc = wrapped


class _SpmdRunner:
    def __init__(self, nc, n_cores):
        _install_verbose_hook()
        self.nc, self.n_cores = nc, n_cores
        pname = nc.partition_id_tensor.name if nc.partition_id_tensor else None
        in_names, out_names, out_avals = [], [], []
        for alloc in nc.m.functions[0].allocations:
            if not isinstance(alloc, mybir.MemoryLocationSet):
                continue
            name = alloc.memorylocations[0].name
            if alloc.kind == "ExternalInput":
                if name != pname:
                    in_names.append(name)
            elif alloc.kind == "ExternalOutput":
                out_names.append(name)
                out_avals.append(jax.core.ShapedArray(
                    tuple(alloc.tensor_shape), mybir.dt.np(alloc.dtype)))
        self.in_names, self.out_names, self.out_avals = in_names, out_names, out_avals
        n_params = len(in_names)
        all_in = list(in_names) + list(out_names)
        if pname is not None:
            all_in.append(pname)

        def _body(*args):
            operands = list(args)
            if pname is not None:
                operands.append(partition_id_tensor())
            return tuple(_bass_exec_p.bind(
                *operands,
                out_avals=tuple(out_avals), in_names=tuple(all_in),
                out_names=tuple(out_names), lowering_input_output_aliases=(),
                sim_require_finite=True, sim_require_nnan=True, nc=nc))

        devices = jax.devices()[:n_cores]
        self.mesh = Mesh(np.asarray(devices), ("core",))
        in_specs = (PartitionSpec("core"),) * (n_params + len(out_names))
        out_specs = (PartitionSpec("core"),) * len(out_names)
        self.fn = jax.jit(shard_map(_body, mesh=self.mesh, in_specs=in_specs,
                                    out_specs=out_specs, check_rep=False),
                          keep_unused=True)
        self._shard = jax.sharding.NamedSharding(self.mesh, PartitionSpec("core"))

    def put_inputs(self, in_maps):
        arrs = []
        for name in self.in_names:
            cat = np.concatenate([np.asarray(m[name]) for m in in_maps], axis=0)
            arrs.append(jax.device_put(cat, self._shard))
        for av in self.out_avals:
            z = np.zeros((self.n_cores * av.shape[0], *av.shape[1:]), av.dtype)
            arrs.append(jax.device_put(z, self._shard))
        return arrs

    def run(self, dev_args):
        outs = self.fn(*dev_args)
        jax.block_until_ready(outs)
        return outs

    def results(self, outs):
        per_core = []
        for c in range(self.n_cores):
            per_core.append({
                name: np.asarray(outs[i]).reshape(
                    self.n_cores, *self.out_avals[i].shape)[c]
                for i, name in enumerate(self.out_names)})
        return per_core


# ---------------------------------------------------------------- host side

def _mask_blocks(mask):
    """Classify transposed 128x512 blocks of the [T,T] mask.

    Returns (block_info, uniq, shifts) where block_info[j][i] is None (all
    masked), -1 (all valid), ("st", slot) (causal staircase valid = p <=
    f - shifts[slot], masked on-device by a bf16 multiply), or ("mk", idx)
    (arbitrary mixed pattern, multiplied from uniq[idx])."""
    m2 = np.asarray(mask).reshape(T, T)
    valid = (m2 != -np.inf)          # [tq, tk]
    validT = valid.T                 # [tk, tq]
    uniq, keys = [], {}
    shifts, shift_keys = [], {}
    p_idx = np.arange(128)[:, None]
    f_idx = np.arange(512)[None, :]
    block_info = []
    for j in range(NJ):
        row = []
        for i in range(NT):
            blk = validT[128 * i:128 * (i + 1), 512 * j:512 * (j + 1)]
            if not blk.any():
                row.append(None)
                continue
            if blk.all():
                row.append(-1)
                continue
            s = 128 * i - 512 * j
            if -512 < s < 512 and np.array_equal(blk, p_idx <= f_idx - s):
                if s not in shift_keys:
                    shift_keys[s] = len(shifts)
                    shifts.append(s)
                row.append(("st", shift_keys[s]))
                continue
            k = hashlib.sha1(np.ascontiguousarray(blk)).hexdigest()
            if k not in keys:
                keys[k] = len(uniq)
                uniq.append(blk.astype(np.float32))
            row.append(("mk", keys[k]))
        block_info.append(row)
    return block_info, uniq, shifts


_CACHE = {}


def _get_runner(block_info, n_uniq, shifts=(), loop_n=None, phases="BCD",
                cast_dma=True):
    key = (str(block_info), n_uniq, tuple(shifts), loop_n, phases)
    if key not in _CACHE:
        nc = _build_nc(block_info, n_uniq, shifts=shifts, loop_n=loop_n,
                       phases=phases, cast_dma=cast_dma)
        _CACHE[key] = _SpmdRunner(nc, N_CORES)
    return _CACHE[key]


def _bf16(a):
    return np.ascontiguousarray(np.asarray(a, np.float32)).astype(
        ml_dtypes.bfloat16)


def _pack_rows(a):
    """[R*128, F] -> [128, R*F]: partition-contiguous packing for fast DMA."""
    r = a.shape[0] // 128
    return np.ascontiguousarray(
        a.reshape(r, 128, a.shape[1]).transpose(1, 0, 2).reshape(128, -1))


def _make_in_maps(x, mask, wq, wk, wv, wo):
    block_info, uniq, shifts = _mask_blocks(mask)
    x = np.asarray(x, np.float32)
    extra = {}
    if uniq:
        mk = np.stack(uniq)    # [u,128,512] -> [128, u*512]
        extra["mk"] = np.ascontiguousarray(
            mk.transpose(1, 0, 2).reshape(128, -1))
    in_maps = []
    for c in range(N_CORES):
        b, g = c // 4, c % 4
        sl = slice(HC * g, HC * (g + 1))
        in_maps.append({
            "xT": _bf16(x[b].T),
            "wqT": _pack_rows(_bf16(np.asarray(wq)[sl, :].T)),
            "wkT": _pack_rows(_bf16(np.asarray(wk)[sl, :].T)),
            "wvT": _pack_rows(_bf16(np.asarray(wv)[sl, :].T)),
            "woT": _pack_rows(_bf16(np.asarray(wo)[:, sl].T)),
            **extra,
        })
    return in_maps, block_info, len(uniq), tuple(shifts)


def kernel(x, mask, wq, wk, wv, wo):
    in_maps, block_info, n_uniq, shifts = _make_in_maps(x, mask, wq, wk, wv, wo)
    runner = _get_runner(block_info, n_uniq, shifts)
    dev = runner.put_inputs(in_maps)
    res = runner.results(runner.run(dev))
    out = np.zeros((B, T, C), np.float32)
    for c in range(N_CORES):
        out[c // 4] += res[c]["y"].astype(np.float32)
    return out


# revision 38
# speedup vs baseline: 1.1593x; 1.1416x over previous
"""Causal multi-head attention (B=2, T=2048, C=1024, H=16, D=64) on 8 trn2 cores.

Sharding: core c -> (batch c//4, head-group c%4 of 4 heads / 256 channels).
Each core computes q/k/v for its head group, causal attention, and a partial
output projection y_part[2048,1024] = attnout_g @ wo_g.T. The host sums the 4
per-group partials of each batch (the "all-reduce after wo" done host-side).

Device kernel (per core, SPMD identical program), bf16 datapath, SOFTWARE
PIPELINED across iterations: q/k/v/x/wo live in two persistent buffer sets;
iteration n runs attention (C/D) on set p while the projections (B) for
iteration n+1 are emitted as PE gap-filler units into set 1-p, interleaved
one unit per attention chunk. Phase C is exp(ACT)-paced, so the B/D filler
matmuls soak up the PE idle the in-order queue would otherwise leave.

  phase B: q^T,k^T packed 2-heads-per-128-partitions (head h in rows
           64*(h%2)..+64 of slot h//2); scores contract K=64 on those
           slices directly (no zero-padding, no scatter copies). v is
           computed NATURALLY (v[t,o] = xT-slice.T @ wvT) into
           vaug[tk, i, h, 0:64] with a ones column at 64.
  phase C: per head/tq-chunk(512): ST[tk128,tq512] = kT.T @ qT (K=64);
           P = exp(ST/8) on ScalarE -> bf16; causal staircase masked on
           P (cols >= 128*rmin only; the rest is never read). PV is
           FLIPPED: out[tq128, 65] = P-slice.T @ vaug (F=65); the 4
           tq-slice accumulation groups run as 2 passes of 2 (one PSUM
           bank per open group - a group's start clobbers other partials
           in the same bank). Column 64 is the softmax denominator,
           per-PARTITION, so normalization is an approx-recip +
           tensor_scalar_mul. attnout natural [tq, ch] is PE-transposed
           back to attnoutT[ch, t] for the out-projection.
  phase D: y = attnoutT.T @ woT per 128-row tile, emitted as filler units.

PSUM banks: scores 2x[128,2,512] (4) + PV/transpose 2x[128,512] (2) +
B/D shared ring 2x[128,512] (2).
"""
import hashlib
import numpy as np
import ml_dtypes
from collections import deque

import jax
import jax.numpy as jnp
from jax.sharding import Mesh, PartitionSpec
from jax.experimental.shard_map import shard_map

import concourse.bass as bass
import concourse.tile as tile
from concourse import bacc, mybir
from concourse import bass2jax
from concourse.bass2jax import _bass_exec_p, install_neuronx_cc_hook, partition_id_tensor

B, T, C = 2, 2048, 1024
H = 16
D = C // H            # 64
SCALE = D ** -0.5     # 0.125
N_CORES = 8
HG = H // (N_CORES // B)   # heads per core = 4
HC = HG * D                # channels per core = 256
KT = C // 128              # 8 contraction tiles
NT = T // 128              # 16 row tiles
NJ = T // 512              # 4 tq tiles
F32 = mybir.dt.float32
BF16 = mybir.dt.bfloat16

import os
K_STAIR = os.environ.get("K_STAIR", "pool")   # pool affine_select | dve mult
K_YDMA = os.environ.get("K_YDMA", "sp")       # y DMA issue queue
LOOKAHEAD = int(os.environ.get("K_LA", "2"))  # chunks in flight exp->PV
PT_BUFS = int(os.environ.get("K_PTB", "10"))  # pt pool depth (2-pass PV)


# ---------------------------------------------------------------- device code

def _build_nc(block_info, n_uniq, shifts=(), loop_n=None, phases="BCVD",
              cast_dma=False, dbg=False):
    """block_info[j][i] = None (skip) | -1 (full) | ("st", slot) (causal
    staircase) | ("mk", idx>=0) (general mask tile multiplied on DVE)."""
    n_shift = len(shifts)
    # per stair slot: first tq-128-slice that reads this tile
    rmin = [max(0, -(-(s - 127) // 128)) for s in shifts]
    use_stm = K_STAIR == "dve" and n_shift > 0
    nc = bacc.Bacc("TRN2", target_bir_lowering=False, debug=False,
                   num_devices=N_CORES)
    xT_ap = nc.dram_tensor("xT", [C, T], BF16, kind="ExternalInput").ap()
    # weights host-packed so each SBUF partition's bytes are contiguous in
    # DRAM (large descriptors; scattered 1KB descriptors measured ~4x slower)
    wqT_ap = nc.dram_tensor("wqT", [128, KT * HC], BF16, kind="ExternalInput").ap()
    wkT_ap = nc.dram_tensor("wkT", [128, KT * HC], BF16, kind="ExternalInput").ap()
    wvT_ap = nc.dram_tensor("wvT", [128, KT * HC], BF16, kind="ExternalInput").ap()
    woT_ap = nc.dram_tensor("woT", [128, 2 * C], BF16, kind="ExternalInput").ap()
    if n_uniq > 0:
        mk_ap = nc.dram_tensor("mk", [128, n_uniq * 512], F32,
                               kind="ExternalInput").ap()
    if use_stm:
        stm_ap = nc.dram_tensor("stm", [128, n_shift * 512], BF16,
                                kind="ExternalInput").ap()
    y_ap = nc.dram_tensor("y", [T, C], BF16, kind="ExternalOutput").ap()
    if dbg:
        dqT_ap = nc.dram_tensor("dqT", [128, 2 * T], BF16, kind="ExternalOutput").ap()
        dkT_ap = nc.dram_tensor("dkT", [128, 2 * T], BF16, kind="ExternalOutput").ap()
        dva_ap = nc.dram_tensor("dva", [128, NT * HG * 66], BF16, kind="ExternalOutput").ap()
        dao_ap = nc.dram_tensor("dao", [128, 2 * T], BF16, kind="ExternalOutput").ap()

    with tile.TileContext(nc) as tc:
        with (
            nc.allow_low_precision(reason="bf16 matmul pipeline"),
            tc.tile_pool(name="glob", bufs=1) as pg,
            tc.tile_pool(name="warm", bufs=1) as pwarm,
            tc.tile_pool(name="wp", bufs=6) as pwp,
            tc.tile_pool(name="ao", bufs=2) as pao,
            tc.tile_pool(name="an", bufs=2) as pan,
            tc.tile_pool(name="mks", bufs=2) as pmks,
            tc.tile_pool(name="pt", bufs=PT_BUFS) as ppt,
            tc.tile_pool(name="small", bufs=8) as psm,
            tc.tile_pool(name="ys", bufs=3) as pys,
            # PSUM: one open accumulation group per 2KB bank (a group's
            # start clobbers other partials sharing its bank).
            tc.tile_pool(name="pst", bufs=2, space="PSUM") as pst,   # 4 banks
            tc.tile_pool(name="ppv", bufs=2, space="PSUM") as ppv,   # 2 banks
            tc.tile_pool(name="pbd", bufs=2, space="PSUM") as pbd,   # 2 banks
        ):
            ident = pg.tile([128, 128], F32)       # PE-transpose identity
            if use_stm:
                stm = pg.tile([128, n_shift, 512], BF16)
                nc.sync.dma_start(
                    stm[:], stm_ap.rearrange("p (s f) -> p s f", f=512))

            # warm the Exp table while DMAs run
            wtile = pwarm.tile([1, 16], F32)
            nc.vector.memset(wtile[:], 0.0)
            nc.scalar.activation(wtile[:], wtile[:],
                                 mybir.ActivationFunctionType.Exp)
            import concourse.masks as _masks
            _masks.make_identity(nc, ident[:])

            # persistent double-buffered projection sets (x single-buffered:
            # its DMA for body n+1 waits body n's last projection read)
            xTs = [pg.tile([128, KT, T], BF16, name="xTs")] * 2
            qTs = [pg.tile([128, 2, T], BF16, name=f"qTs{s}") for s in range(2)]
            kTs = [pg.tile([128, 2, T], BF16, name=f"kTs{s}") for s in range(2)]
            vas = [pg.tile([128, NT, HG, 66], BF16, name=f"vas{s}") for s in range(2)]
            wos = [pg.tile([128, 2, C], BF16, name=f"wos{s}") for s in range(2)]

            def dma_set(s, it):
                """Issue input DMAs for projection set s."""
                xT = xTs[s]
                xr = xT_ap.rearrange("(k p) t -> k p t", p=128)
                w_ts = [pwp.tile([128, KT, HC], BF16, tag="w",
                                 name=f"w{qk}_{it}") for qk in range(3)]
                wqr = wqT_ap.rearrange("p (k m) -> p k m", k=KT)
                nc.sync.dma_start(w_ts[0][:, 0:KT // 2, :], wqr[:, 0:KT // 2, :])
                nc.sync.dma_start(xT[:, 0, :], xr[0])
                nc.sync.dma_start(w_ts[0][:, KT // 2:, :], wqr[:, KT // 2:, :])
                nc.sync.dma_start(xT[:, 1, :], xr[1])
                nc.sync.dma_start(w_ts[1][:], wkT_ap.rearrange("p (k m) -> p k m", k=KT))
                for kc in range(2, KT):
                    nc.sync.dma_start(xT[:, kc, :], xr[kc])
                nc.sync.dma_start(w_ts[2][:], wvT_ap.rearrange("p (k m) -> p k m", k=KT))
                nc.sync.dma_start(wos[s][:], woT_ap.rearrange("p (k m) -> p k m", k=2))
                return w_ts

            def b_units(s, w_ts, it):
                """Projection work for set s as a list of filler closures."""
                units = []
                if "B" not in phases:
                    return units
                xT, qT, kTp, vaug = xTs[s], qTs[s], kTs[s], vas[s]
                for qk in range(2):
                    dst = qT if qk == 0 else kTp
                    for m in range(2):
                        for j in range(4):
                            def u(qk=qk, m=m, j=j, dst=dst):
                                pss = pbd.tile([128, 512], F32, tag="bd",
                                               name=f"qk{qk}_{m}_{j}_{it}")
                                for kc in range(KT):
                                    nc.tensor.matmul(
                                        pss[:],
                                        w_ts[qk][:, kc, 128 * m:128 * (m + 1)],
                                        xT[:, kc, 512 * j:512 * (j + 1)],
                                        start=(kc == 0), stop=(kc == KT - 1))
                                nc.vector.tensor_copy(
                                    dst[:, m, 512 * j:512 * (j + 1)], pss[:])
                            units.append(u)
                for i in range(NT):
                    def u(i=i):
                        pvn = pbd.tile([128, 512], F32, tag="bd",
                                       name=f"vn{i}_{it}")
                        flat = pvn[:, 0:HC]
                        for kc in range(KT):
                            nc.tensor.matmul(
                                flat, xT[:, kc, 128 * i:128 * (i + 1)],
                                w_ts[2][:, kc, :],
                                start=(kc == 0), stop=(kc == KT - 1))
                        nc.vector.tensor_copy(
                            vaug[:, i, :, 0:D],
                            flat.rearrange("p (h d) -> p h d", h=HG))
                    units.append(u)
                units.append(lambda: nc.vector.memset(vaug[:, :, :, D], 1.0))
                return units

            def body(it, rd, wr):
                """Attention on set rd; fillers project into set wr."""
                fillers = deque()
                if wr is not None:
                    w_ts = dma_set(wr, it)
                if n_uniq > 0:
                    mks = pmks.tile([128, n_uniq, 512], F32, tag="mks",
                                    name=f"mks_{it}")
                    nc.sync.dma_start(mks[:], mk_ap.rearrange("p (u f) -> p u f", f=512))
                deferred = []
                if wr is not None:
                    if wr == rd:
                        # non-pipelined: projections must run after attention
                        deferred = b_units(wr, w_ts, it)
                    else:
                        fillers.extend(b_units(wr, w_ts, it))
                if "C" not in phases:
                    for u in (*fillers, *deferred):
                        u()
                    return
                qT, kTp, vaug = qTs[rd], kTs[rd], vas[rd]
                woT = wos[rd]
                attnoutT = pao.tile([128, 2, T], BF16, tag="ao", name=f"ao_{it}")

                def d_unit(j, tp, tsub):
                    def emit():
                        t = 4 * j + 2 * tp + tsub
                        ys = d_unit.ys
                        if tsub == 0:
                            ys = d_unit.ys = pys.tile(
                                [128, 2, C], BF16, tag="ys", name=f"ys{t}_{it}")
                        for o2 in range(2):
                            yps = pbd.tile([128, 512], F32, tag="bd",
                                           name=f"yps{t}_{o2}_{it}")
                            for kc in range(2):
                                nc.tensor.matmul(
                                    yps[:],
                                    attnoutT[:, kc, 128 * t:128 * (t + 1)],
                                    woT[:, kc, 512 * o2:512 * (o2 + 1)],
                                    start=(kc == 0), stop=(kc == 1))
                            nc.vector.tensor_copy(
                                ys[:, tsub, 512 * o2:512 * (o2 + 1)], yps[:])
                        if tsub == 1:
                            r0 = 512 * j + 256 * tp
                            eng = {"pool": nc.gpsimd, "act": nc.scalar,
                                   "sp": nc.sync}[K_YDMA]
                            eng.dma_start(
                                y_ap[r0:r0 + 256, :].rearrange(
                                    "(tt p) o -> p tt o", p=128),
                                ys[:])
                    return emit
                d_unit.ys = None

                for j in range(NJ):
                    blocks = [(i, bi) for i, bi in enumerate(block_info[j])
                              if bi is not None]
                    chunks = [blocks[c:c + 2] for c in range(0, len(blocks), 2)]
                    # PV bookkeeping: contributors per tq-slice
                    contrib = [[] for _ in range(4)]
                    for i, bi in blocks:
                        lo = rmin[bi[1]] if (isinstance(bi, tuple)
                                             and bi[0] == "st") else 0
                        for rp in range(lo, 4):
                            contrib[rp].append(i)
                    anat = pan.tile([128, 2, 4, 2, D], F32, tag="an",
                                    name=f"an{j}_{it}")
                    for h in range(HG):
                        m, hh = h // 2, h % 2
                        r0 = 64 * hh
                        jsl = slice(512 * j, 512 * (j + 1))
                        seen = [0] * 4
                        pvs = [None] * 4

                        def emit_pv(pt, ch, rps):
                            if "V" not in phases:
                                return
                            for c, (i, bi) in enumerate(ch):
                                lo = rmin[bi[1]] if (isinstance(bi, tuple)
                                                     and bi[0] == "st") else 0
                                for rp in rps:
                                    if rp < lo:
                                        continue
                                    seen[rp] += 1
                                    nc.tensor.matmul(
                                        pvs[rp][:, 0:65],
                                        pt[:, c, 128 * rp:128 * (rp + 1)],
                                        vaug[:, i, h, 0:65],
                                        start=(seen[rp] == 1),
                                        stop=(seen[rp] == len(contrib[rp])))

                        def norm(rp):
                            if not contrib[rp]:
                                nc.vector.memset(anat[:, m, rp, hh, :], 0.0)
                                return
                            dn = psm.tile([128, 1], F32, tag="dn")
                            nc.vector.tensor_copy(dn[:], pvs[rp][:, D:D + 1])
                            rc = psm.tile([128, 1], F32, tag="rc")
                            nc.vector.reciprocal_approx_fast(rc[:], dn[:])
                            nc.vector.tensor_scalar_mul(
                                anat[:, m, rp, hh, :], pvs[rp][:, 0:D], rc[:])

                        # pass 1: tq-slices 0,1 accumulate while chunks flow
                        if "V" in phases:
                            for rp in (0, 1):
                                if contrib[rp]:
                                    pvs[rp] = ppv.tile(
                                        [128, 512], F32, tag="pv",
                                        name=f"pv{h}_{j}_{rp}_{it}")
                        pend = deque()
                        pts = []
                        for ch in chunks:
                            nsub = len(ch)
                            st = pst.tile([128, 2, 512], F32, tag="st",
                                          name=f"st{h}_{j}_{it}")
                            for c, (i, bi) in enumerate(ch):
                                nc.tensor.matmul(
                                    st[:, c, :],
                                    kTp[r0:r0 + 64, m, 128 * i:128 * (i + 1)],
                                    qT[r0:r0 + 64, m, jsl],
                                    start=True, stop=True)
                            pt = ppt.tile([128, 2, 512], BF16, tag="pt")
                            # one exp per chunk: fixed per-op ACT cost
                            # amortizes over 1024 elements
                            nc.scalar.activation(
                                pt[:, 0:nsub, :], st[:, 0:nsub, :],
                                mybir.ActivationFunctionType.Exp, scale=SCALE)
                            for c, (i, bi) in enumerate(ch):
                                if not isinstance(bi, tuple):
                                    continue
                                if bi[0] == "st":
                                    # zero exp output where p > f - s
                                    # (cols < 128*rmin are never read)
                                    s = shifts[bi[1]]
                                    c0 = 128 * rmin[bi[1]]
                                    if use_stm:
                                        nc.vector.tensor_mul(
                                            pt[:, c, c0:], pt[:, c, c0:],
                                            stm[:, bi[1], c0:])
                                    else:
                                        nc.gpsimd.affine_select(
                                            out=pt[:, c, c0:],
                                            in_=pt[:, c, c0:],
                                            compare_op=mybir.AluOpType.is_ge,
                                            fill=0.0,
                                            base=c0 - s,
                                            pattern=[[1, 512 - c0]],
                                            channel_multiplier=-1)
                            pend.append((pt, ch))
                            pts.append((pt, ch))
                            if len(pend) > LOOKAHEAD:
                                emit_pv(*pend.popleft(), (0, 1))
                            if fillers:
                                fillers.popleft()()
                        while pend:
                            emit_pv(*pend.popleft(), (0, 1))
                        if "V" not in phases:
                            continue
                        norm(0)
                        norm(1)
                        # pass 2: tq-slices 2,3 re-sweep the kept pt tiles
                        for rp in (2, 3):
                            if contrib[rp]:
                                pvs[rp] = ppv.tile(
                                    [128, 512], F32, tag="pv",
                                    name=f"pv{h}_{j}_{rp}_{it}")
                        for pt, ch in pts:
                            emit_pv(pt, ch, (2, 3))
                        norm(2)
                        norm(3)

                        if hh == 1:
                            # both heads of pair m done: transpose natural
                            # attnout back to channel-major for out-proj
                            for half in range(2):
                                trp = ppv.tile([128, 512], F32, tag="pv",
                                               name=f"tr{m}_{half}_{j}_{it}")
                                for q2 in range(2):
                                    rp = 2 * half + q2
                                    nc.tensor.transpose(
                                        trp[:, 128 * q2:128 * (q2 + 1)],
                                        anat[:, m, rp, :, :].rearrange(
                                            "p a b -> p (a b)"),
                                        ident[:])
                                a0 = 512 * j + 256 * half
                                nc.vector.tensor_copy(
                                    attnoutT[:, m, a0:a0 + 256],
                                    trp[:, 0:256])

                    if "D" in phases and "V" in phases:
                        for tp in range(2):
                            for tsub in range(2):
                                fillers.append(d_unit(j, tp, tsub))
                while fillers:
                    fillers.popleft()()
                for u in deferred:
                    u()
                if dbg:
                    nc.sync.dma_start(dqT_ap.rearrange("p (a t) -> p a t", a=2), qT[:])
                    nc.sync.dma_start(dkT_ap.rearrange("p (a t) -> p a t", a=2), kTp[:])
                    nc.sync.dma_start(dva_ap.rearrange("p (i h e) -> p i h e", i=NT, h=HG), vaug[:])
                    nc.sync.dma_start(dao_ap.rearrange("p (a t) -> p a t", a=2), attnoutT[:])

            def prologue():
                w_ts = dma_set(0, "p")
                for u in b_units(0, w_ts, "p"):
                    u()

            prologue()
            if loop_n is None:
                body(0, 0, None)
            elif loop_n % 2:
                # odd loop count: non-pipelined fallback, single set
                with tc.For_i(0, loop_n, 1, staggered_reset=True):
                    body(0, 0, 0)
            else:
                unroll = 4 if loop_n % 4 == 0 else 2
                with tc.For_i(0, loop_n // unroll, 1, staggered_reset=True):
                    for it in range(unroll):
                        body(it, it % 2, 1 - it % 2)

    nc.compile()
    return nc


# ---------------------------------------------------------------- run harness

def _install_verbose_hook():
    install_neuronx_cc_hook()
    try:
        import libneuronxla
    except ImportError:
        return
    import traceback
    inner = bass2jax.neuronx_cc_hook

    def wrapped(*a, **kw):
        try:
            return inner(*a, **kw)
        except BaseException:
            traceback.print_exc()
            raise
    libneuronxla.neuronx_cc = wrapped


class _SpmdRunner:
    def __init__(self, nc, n_cores):
        _install_verbose_hook()
        self.nc, self.n_cores = nc, n_cores
        pname = nc.partition_id_tensor.name if nc.partition_id_tensor else None
        in_names, out_names, out_avals = [], [], []
        for alloc in nc.m.functions[0].allocations:
            if not isinstance(alloc, mybir.MemoryLocationSet):
                continue
            name = alloc.memorylocations[0].name
            if alloc.kind == "ExternalInput":
                if name != pname:
                    in_names.append(name)
            elif alloc.kind == "ExternalOutput":
                out_names.append(name)
                out_avals.append(jax.core.ShapedArray(
                    tuple(alloc.tensor_shape), mybir.dt.np(alloc.dtype)))
        self.in_names, self.out_names, self.out_avals = in_names, out_names, out_avals
        n_params = len(in_names)
        all_in = list(in_names) + list(out_names)
        if pname is not None:
            all_in.append(pname)

        def _body(*args):
            operands = list(args)
            if pname is not None:
                operands.append(partition_id_tensor())
            return tuple(_bass_exec_p.bind(
                *operands,
                out_avals=tuple(out_avals), in_names=tuple(all_in),
                out_names=tuple(out_names), lowering_input_output_aliases=(),
                sim_require_finite=True, sim_require_nnan=True, nc=nc))

        devices = jax.devices()[:n_cores]
        self.mesh = Mesh(np.asarray(devices), ("core",))
        in_specs = (PartitionSpec("core"),) * (n_params + len(out_names))
        out_specs = (PartitionSpec("core"),) * len(out_names)
        self.fn = jax.jit(shard_map(_body, mesh=self.mesh, in_specs=in_specs,
                                    out_specs=out_specs, check_rep=False),
                          keep_unused=True)
        self._shard = jax.sharding.NamedSharding(self.mesh, PartitionSpec("core"))

    def put_inputs(self, in_maps):
        arrs = []
        for name in self.in_names:
            cat = np.concatenate([np.asarray(m[name]) for m in in_maps], axis=0)
            arrs.append(jax.device_put(cat, self._shard))
        for av in self.out_avals:
            z = np.zeros((self.n_cores * av.shape[0], *av.shape[1:]), av.dtype)
            arrs.append(jax.device_put(z, self._shard))
        return arrs

    def run(self, dev_args):
        outs = self.fn(*dev_args)
        jax.block_until_ready(outs)
        return outs

    def results(self, outs):
        per_core = []
        for c in range(self.n_cores):
            per_core.append({
                name: np.asarray(outs[i]).reshape(
                    self.n_cores, *self.out_avals[i].shape)[c]
                for i, name in enumerate(self.out_names)})
        return per_core


# ---------------------------------------------------------------- host side

def _mask_blocks(mask):
    """Classify transposed 128x512 blocks of the [T,T] mask.

    Returns (block_info, uniq, shifts) where block_info[j][i] is None (all
    masked), -1 (all valid), ("st", slot) (causal staircase valid = p <=
    f - shifts[slot]), or ("mk", idx) (arbitrary pattern from uniq[idx])."""
    m2 = np.asarray(mask).reshape(T, T)
    valid = (m2 != -np.inf)          # [tq, tk]
    validT = valid.T                 # [tk, tq]
    uniq, keys = [], {}
    shifts, shift_keys = [], {}
    p_idx = np.arange(128)[:, None]
    f_idx = np.arange(512)[None, :]
    block_info = []
    for j in range(NJ):
        row = []
        for i in range(NT):
            blk = validT[128 * i:128 * (i + 1), 512 * j:512 * (j + 1)]
            if not blk.any():
                row.append(None)
                continue
            if blk.all():
                row.append(-1)
                continue
            s = 128 * i - 512 * j
            if -512 < s < 512 and np.array_equal(blk, p_idx <= f_idx - s):
                if s not in shift_keys:
                    shift_keys[s] = len(shifts)
                    shifts.append(s)
                row.append(("st", shift_keys[s]))
                continue
            k = hashlib.sha1(np.ascontiguousarray(blk)).hexdigest()
            if k not in keys:
                keys[k] = len(uniq)
                uniq.append(blk.astype(np.float32))
            row.append(("mk", keys[k]))
        block_info.append(row)
    return block_info, uniq, shifts


_CACHE = {}


def _get_runner(block_info, n_uniq, shifts=(), loop_n=None, phases="BCVD",
                cast_dma=True):
    key = (str(block_info), n_uniq, tuple(shifts), loop_n, phases,
           K_STAIR, K_YDMA, LOOKAHEAD, PT_BUFS)
    if key not in _CACHE:
        nc = _build_nc(block_info, n_uniq, shifts=shifts, loop_n=loop_n,
                       phases=phases, cast_dma=cast_dma)
        _CACHE[key] = _SpmdRunner(nc, N_CORES)
    return _CACHE[key]


def _bf16(a):
    return np.ascontiguousarray(np.asarray(a, np.float32)).astype(
        ml_dtypes.bfloat16)


def _pack_rows(a):
    """[R*128, F] -> [128, R*F]: partition-contiguous packing for fast DMA."""
    r = a.shape[0] // 128
    return np.ascontiguousarray(
        a.reshape(r, 128, a.shape[1]).transpose(1, 0, 2).reshape(128, -1))


def _make_in_maps(x, mask, wq, wk, wv, wo):
    block_info, uniq, shifts = _mask_blocks(mask)
    x = np.asarray(x, np.float32)
    extra = {}
    if uniq:
        mk = np.stack(uniq)    # [u,128,512] -> [128, u*512]
        extra["mk"] = np.ascontiguousarray(
            mk.transpose(1, 0, 2).reshape(128, -1))
    if shifts and K_STAIR == "dve":
        p_idx = np.arange(128)[:, None]
        f_idx = np.arange(512)[None, :]
        stm = np.stack([(p_idx <= f_idx - s).astype(np.float32)
                        for s in shifts])          # [s,128,512]
        extra["stm"] = _bf16(np.ascontiguousarray(
            stm.transpose(1, 0, 2).reshape(128, -1)))
    in_maps = []
    for c in range(N_CORES):
        b, g = c // 4, c % 4
        sl = slice(HC * g, HC * (g + 1))
        in_maps.append({
            "xT": _bf16(x[b].T),
            "wqT": _pack_rows(_bf16(np.asarray(wq)[sl, :].T)),
            "wkT": _pack_rows(_bf16(np.asarray(wk)[sl, :].T)),
            "wvT": _pack_rows(_bf16(np.asarray(wv)[sl, :].T)),
            "woT": _pack_rows(_bf16(np.asarray(wo)[:, sl].T)),
            **extra,
        })
    return in_maps, block_info, len(uniq), tuple(shifts)


def kernel(x, mask, wq, wk, wv, wo):
    in_maps, block_info, n_uniq, shifts = _make_in_maps(x, mask, wq, wk, wv, wo)
    runner = _get_runner(block_info, n_uniq, shifts)
    dev = runner.put_inputs(in_maps)
    res = runner.results(runner.run(dev))
    out = np.zeros((B, T, C), np.float32)
    for c in range(N_CORES):
        out[c // 4] += res[c]["y"].astype(np.float32)
    return out
